# revision 11
# baseline (speedup 1.0000x reference)
# Multi-level GNN (3-level GraphConv encoder + sparse interpolation + 3 MLP
# decoders) on 8 Trainium2 NeuronCores.
#
# Sharding: nodes of every level are partitioned contiguously across the 8
# cores; edges and sparse-A rows are sharded by destination node and sorted by
# destination. Neighbor features are fetched with dma_gather row gathers from
# replicated DRAM tables (replication via AllGather collectives), and the
# per-destination segment sums run on the TensorEngine as indicator matmuls.
# Tables/hidden activations use bf16 storage; accumulation stays fp32 in PSUM.
import os
import sys
import numpy as np

for _p in ("/opt/trn_rl_repo", "/root/.axon_site/_ro/trn_rl_repo"):
    if _p not in sys.path and os.path.isdir(_p):
        sys.path.insert(0, _p)

from contextlib import ExitStack

import ml_dtypes

import concourse.bass as bass
import concourse.bacc as bacc
import concourse.tile as tile
import concourse.mybir as mybir
from concourse import bass_utils

F32 = mybir.dt.float32
BF16 = mybir.dt.bfloat16
I16 = mybir.dt.int16
AF = mybir.ActivationFunctionType
ALU = mybir.AluOpType
NPBF = ml_dtypes.bfloat16

C = 8          # cores
H = 128        # hidden
FIN = 6        # input feature dim
F3 = 96        # 3*F
DEC = 64       # decoder width
BLK = 128      # destination block (PSUM partition tile)
ELEM = 128     # table row width (bf16 -> 256B rows, dma_gather minimum)
B0 = 32768     # int16 source-bucket split for level-0 tables

# level sizes: (N, S) with S = per-core shard (C*S >= N)
LEVELS = [(50000, 6250), (12500, 1568), (3125, 392)]
NCHUNK = 512   # node chunk for dense matmuls
KCH = 32       # gather chunk size in 128-edge groups

WPF_L = 259    # f32 weight-pack columns per level
WPB_L = 352    # bf16 weight-pack columns per level


# --------------------------------------------------------------------------
# host-side preprocessing (index manipulation only)
# --------------------------------------------------------------------------

def _wrap16(flat, G):
    """dma_gather index layout: element g*128+p is read from
    idx[p % 16, 8*g + p//16]; tile the 16 rows to 128 partitions."""
    a = flat.reshape(G, 8, 16)
    i16 = a.transpose(2, 0, 1).reshape(16, G * 8)
    return np.tile(i16, (8, 1)).astype(np.int16)


def _build_sched(src, dst, vals, S, NB, C_, buckets):
    """Shard (src,dst[,vals]) by destination shard of size S, sort by dst,
    split by source-row bucket, pad each 128-dst block's edge list per bucket
    to a multiple of 128 with a max-over-cores schedule.

    Returns a list over passes (one per bucket) of dicts:
      ng   tuple[NB]           groups per block
      i16  [C, 128, 8G] int16  wrapped gather ids (bucket-relative; pad=0)
      dl   [C, 128, G]  f32    dst local id in block (pad=999)
      vl   [C, 128, G]  f32    optional vals (pad=0)
    """
    percore = []
    for c in range(C_):
        lo, hi = c * S, (c + 1) * S
        m = (dst >= lo) & (dst < hi)
        s = src[m].astype(np.int64)
        d = (dst[m] - lo).astype(np.int64)
        v = vals[m].astype(np.float32) if vals is not None else None
        o = np.argsort(d, kind="stable")
        s, d = s[o], d[o]
        if v is not None:
            v = v[o]
        percore.append((s, d, v))

    passes = []
    for pi, (blo, bhi) in enumerate(buckets):
        pc = []
        for (s, d, v) in percore:
            m = (s >= blo) & (s < bhi)
            sb, db = s[m] - blo, d[m]
            vb = v[m] if v is not None else None
            counts = np.bincount(db >> 7, minlength=NB)
            pc.append((sb, db, vb, counts))
        ng = np.zeros(NB, np.int64)
        if pi == 0:
            ng[:] = 1  # ensure at least one (padded) group: zeroes the block
        for (_, _, _, counts) in pc:
            ng = np.maximum(ng, (counts + 127) // 128)
        G = int(ng.sum())
        goff = np.zeros(NB + 1, np.int64)
        goff[1:] = np.cumsum(ng)
        i16 = np.zeros((C_, 128, 8 * G), np.int16)
        dl = np.full((C_, 128, G), 999.0, np.float32)
        vl = np.zeros((C_, 128, G), np.float32) if vals is not None else None
        for c, (sb, db, vb, counts) in enumerate(pc):
            coff = np.zeros(NB + 1, np.int64)
            coff[1:] = np.cumsum(counts)
            flat_i = np.zeros((G, 128), np.int64)
            flat_d = np.full((G, 128), 999.0, np.float32)
            flat_v = np.zeros((G, 128), np.float32) if vals is not None else None
            for b in range(NB):
                if ng[b] == 0:
                    continue
                e0, e1 = int(coff[b]), int(coff[b + 1])
                n = e1 - e0
                g0 = int(goff[b])
                L = int(ng[b]) * 128
                ai = np.zeros(L, np.int64)
                ad = np.full(L, 999.0, np.float32)
                ai[:n] = sb[e0:e1]
                ad[:n] = (db[e0:e1] & 127).astype(np.float32)
                flat_i[g0:g0 + int(ng[b])] = ai.reshape(-1, 128)
                flat_d[g0:g0 + int(ng[b])] = ad.reshape(-1, 128)
                if vals is not None:
                    av = np.zeros(L, np.float32)
                    av[:n] = vb[e0:e1]
                    flat_v[g0:g0 + int(ng[b])] = av.reshape(-1, 128)
            i16[c] = _wrap16(flat_i, G)
            dl[c] = flat_d.T
            if vals is not None:
                vl[c] = flat_v.T
        passes.append({"ng": tuple(int(x) for x in ng), "i16": i16,
                       "dl": dl, "vl": vl})
    return passes


def preprocess(inputs, levels=None):
    levels = levels or LEVELS
    f32 = np.float32
    xs = [np.asarray(inputs[f"x{l}"], f32) for l in range(3)]
    eis = [np.asarray(inputs[f"edge_index{l}"]).astype(np.int64) for l in range(3)]
    Ar = [np.asarray(inputs[f"A{m}_rows"]).astype(np.int64) for m in (1, 2)]
    Ac = [np.asarray(inputs[f"A{m}_cols"]).astype(np.int64) for m in (1, 2)]
    Av = [np.asarray(inputs[f"A{m}_vals"], f32) for m in (1, 2)]

    meta = {"levels": [], "interp": [], "ioff": {}, "foff": {}}
    in_maps = [dict() for _ in range(C)]

    def put(name, arrs):
        if isinstance(arrs, np.ndarray):
            arrs = [arrs] * C
        for c in range(C):
            in_maps[c][name] = np.ascontiguousarray(arrs[c])

    put("identity", np.eye(128, dtype=f32))
    put("identb", np.eye(128, dtype=f32).astype(NPBF))
    put("iota", np.tile(np.arange(128, dtype=f32), (128, 1)))

    ipacks = [[] for _ in range(C)]   # int16 [128, *]
    fpacks = [[] for _ in range(C)]   # f32   [128, *]
    icol = [0]
    fcol = [0]

    def add_pass(name, p):
        G8 = p["i16"].shape[2]
        meta["ioff"][name] = (icol[0], G8)
        icol[0] += G8
        G = p["dl"].shape[2]
        meta["foff"][name] = (fcol[0], G)
        fcol[0] += G
        for c in range(C):
            ipacks[c].append(p["i16"][c])
            fpacks[c].append(p["dl"][c])
        if p["vl"] is not None:
            meta["foff"][name + "v"] = (fcol[0], G)
            fcol[0] += G
            for c in range(C):
                fpacks[c].append(p["vl"][c])

    wpf = np.zeros((128, 3 * WPF_L), f32)
    wpb = np.zeros((128, 3 * WPB_L), f32)

    for l, (N, S) in enumerate(levels):
        NB = (S + BLK - 1) // BLK
        buckets = [(0, B0), (B0, N)] if N > B0 else [(0, N)]
        passes = _build_sched(eis[l][0], eis[l][1], None, S, NB, C, buckets)
        meta["levels"].append({
            "N": N, "S": S, "NB": NB,
            "ngs": [p["ng"] for p in passes],
            "buckets": buckets,
        })
        for pi, p in enumerate(passes):
            add_pass(f"e{l}p{pi}", p)
        xt = np.zeros((N, ELEM), f32)
        xt[:, :FIN] = xs[l]
        put(f"xtab{l}", xt.astype(NPBF))
        xT = np.zeros((C, FIN, S), f32)
        for c in range(C):
            lo = c * S
            hi = min(N, lo + S)
            if hi > lo:
                xT[c, :, : hi - lo] = xs[l][lo:hi].T
        put(f"xT{l}", [xT[c] for c in range(C)])

        Lf, Lb = l * WPF_L, l * WPB_L
        wpf[:, Lf + 0] = np.asarray(inputs["enc_c1_brel"][l], f32)
        wpf[:, Lf + 1] = np.asarray(inputs["enc_c2_brel"][l], f32)
        wpf[:F3, Lf + 2] = np.asarray(inputs["enc_lin_b"][l], f32)
        wpf[:FIN, Lf + 3:Lf + 131] = np.asarray(inputs["enc_c1_Wrel"][l], f32)
        wpf[:FIN, Lf + 131:Lf + 259] = np.asarray(inputs["enc_c1_Wroot"][l], f32)
        wpb[:, Lb:Lb + 128] = np.asarray(inputs["enc_c2_Wrel"][l], f32)
        wpb[:, Lb + 128:Lb + 256] = np.asarray(inputs["enc_c2_Wroot"][l], f32)
        wpb[:, Lb + 256:Lb + 352] = np.asarray(inputs["enc_lin_W"][l], f32)

    put("wpf", wpf)
    put("wpb", wpb.astype(NPBF))

    S0 = levels[0][1]
    NB0 = (S0 + BLK - 1) // BLK
    for m in (1, 2):
        passes = _build_sched(Ac[m - 1], Ar[m - 1], Av[m - 1], S0, NB0, C,
                              [(0, levels[m][0])])
        meta["interp"].append({"ngs": [p["ng"] for p in passes]})
        add_pass(f"a{m}p0", passes[0])

    put("ipack", [np.concatenate(ipacks[c], axis=1) for c in range(C)])
    put("fpack", [np.concatenate(fpacks[c], axis=1) for c in range(C)])

    # decoder weights, regrouped so branch b sits on partitions [32b, 32b+32)
    W0 = np.asarray(inputs["dec_W0"], f32)      # [3, 96, 64]
    b0 = np.asarray(inputs["dec_b0"], f32)      # [3, 64]
    Wout = np.asarray(inputs["dec_Wout"], f32)  # [3, 64, 1]
    bout = np.asarray(inputs["dec_bout"], f32)  # [3, 1]
    wdecb = np.zeros((128, 196), f32)
    for j in range(3):
        for b in range(3):
            wdecb[32 * b:32 * b + 32, 64 * j:64 * j + 64] = W0[b, 32 * j:32 * j + 32, :]
    wdecb[:DEC, 192:195] = Wout[:, :, 0].T
    put("wdecb", wdecb.astype(NPBF))
    wdecf = np.zeros((DEC, 6), f32)
    wdecf[:, 0:3] = b0.T
    # ELU is computed as relu(x)+exp(min(x,0)); the missing "-1" is folded
    # into the output bias: bout' = bout - sum_k Wout[k].
    wdecf[0, 3:6] = bout[:, 0] - Wout[:, :, 0].sum(axis=1)
    put("wdecf", wdecf)

    return meta, in_maps


# --------------------------------------------------------------------------
# device program
# --------------------------------------------------------------------------

def _seg_agg(tc, nc, name, passes, S, fuse, outT, iota_ap, kch):
    """Segment-sum gather-aggregate.  outT[:fuse, d] = sum over edges with
    dst==d of table[src, :fuse] (times val when given).  outT transposed:
    [fuse partitions, S free].  passes: list of dicts with keys
    tab (DRAM AP [rows, ELEM] bf16), ei (int16 [128, 8G]), ed (f32 [128, G]),
    av (f32 [128, G] or None), ng (tuple), accum (bool)."""
    with tc.tile_pool(name=f"g_{name}", bufs=2) as gp, \
         tc.tile_pool(name=f"s_{name}", bufs=3) as sp, \
         tc.tile_pool(name=f"p_{name}", bufs=2, space="PSUM") as pp:
        for P in passes:
            ng = P["ng"]
            G = int(sum(ng))
            g = 0
            gt = None
            gt_lo = gt_hi = 0
            for b, nb in enumerate(ng):
                if nb == 0:
                    continue
                wb = min(BLK, S - BLK * b)
                ps = pp.tile([fuse, BLK], F32, tag="aggps", name="aggps")
                for k in range(nb):
                    if g >= gt_hi:
                        c0 = g
                        c1 = min(g + kch, G)
                        n = (c1 - c0) * 128
                        gt = gp.tile([128, c1 - c0, ELEM], BF16, tag="gt",
                                     name="gt")
                        nc.gpsimd.dma_gather(
                            gt[:, :, :], P["tab"], P["ei"][:, 8 * c0:8 * c1],
                            n, n, ELEM, single_packet=False)
                        gt_lo, gt_hi = c0, c1
                    st = sp.tile([128, 128], BF16, tag="st", name="st")
                    if P["av"] is not None:
                        nc.vector.tensor_scalar(
                            out=st[:], in0=iota_ap, scalar1=P["ed"][:, g:g + 1],
                            scalar2=P["av"][:, g:g + 1], op0=ALU.is_equal,
                            op1=ALU.mult)
                    else:
                        nc.vector.tensor_scalar(
                            out=st[:], in0=iota_ap, scalar1=P["ed"][:, g:g + 1],
                            scalar2=None, op0=ALU.is_equal)
                    nc.tensor.matmul(ps[:, :wb], lhsT=gt[:, g - gt_lo, :fuse],
                                     rhs=st[:, :wb], start=(k == 0),
                                     stop=(k == nb - 1))
                    g += 1
                sl = outT[:, BLK * b:BLK * b + wb]
                if P["accum"]:
                    nc.vector.tensor_tensor(out=sl, in0=ps[:, :wb], in1=sl,
                                            op=ALU.add)
                else:
                    nc.scalar.activation(sl, ps[:, :wb], AF.Copy)


def _dense2(tc, nc, name, aggT, otherT, wa, wb_, bias, kin, fout, S, act, outT):
    """outT[:fout, n] = act(wa.T @ aggT + wb_.T @ otherT + bias)"""
    with tc.tile_pool(name=f"dp_{name}", bufs=2, space="PSUM") as pp:
        for n0 in range(0, S, NCHUNK):
            w = min(NCHUNK, S - n0)
            ps = pp.tile([fout, NCHUNK], F32, tag="ps", name="ps")
            nc.tensor.matmul(ps[:, :w], lhsT=wa[:kin, :fout], rhs=aggT[:kin, n0:n0 + w],
                             start=True, stop=(otherT is None))
            if otherT is not None:
                nc.tensor.matmul(ps[:, :w], lhsT=wb_[:kin, :fout],
                                 rhs=otherT[:kin, n0:n0 + w], start=False, stop=True)
            nc.scalar.activation(outT[:fout, n0:n0 + w], ps[:, :w], act, bias=bias)


def _write_rows(tc, nc, name, srcT, fdim, S, bounce, ident, dt_out, width=None):
    """DRAM bounce[r, :fdim] = srcT[:, r] via PE transpose (cast to dt_out).
    When width > fdim, columns [fdim, width) are zero-filled."""
    dt_in = srcT.dtype
    width = width or fdim
    with tc.tile_pool(name=f"wp_{name}", bufs=2, space="PSUM") as pp, \
         tc.tile_pool(name=f"ws_{name}", bufs=3) as sp:
        for c0 in range(0, S, 128):
            w = min(128, S - c0)
            pt = pp.tile([128, fdim], dt_in, tag="pt", name="pt")
            nc.tensor.transpose(pt[:w, :], in_=srcT[:fdim, c0:c0 + w],
                                identity=ident[:fdim, :fdim])
            st = sp.tile([128, width], dt_out, tag="st", name="st")
            if width > fdim:
                nc.vector.memset(st[:, fdim:width], 0.0)
            nc.scalar.activation(st[:w, :fdim], pt[:w, :], AF.Copy)
            nc.sync.dma_start(bounce[c0:c0 + w, :width], st[:w, :width])


def emit(tc, outs, ins, meta):
    nc = tc.nc
    rg = [list(range(C))]
    lv = meta["levels"]
    S0 = lv[0]["S"]

    with ExitStack() as ctx:
        const = ctx.enter_context(tc.tile_pool(name="const", bufs=1))
        ident = const.tile([128, 128], F32)
        nc.sync.dma_start(ident[:], ins["identity"][:])
        identb = const.tile([128, 128], BF16)
        nc.sync.dma_start(identb[:], ins["identb"][:])
        iota = const.tile([128, 128], F32)
        nc.sync.dma_start(iota[:], ins["iota"][:])
        wpf = const.tile([128, 3 * WPF_L], F32)
        nc.sync.dma_start(wpf[:], ins["wpf"][:])
        wpb = const.tile([128, 3 * WPB_L], BF16)
        nc.sync.dma_start(wpb[:], ins["wpb"][:])
        wdecb = const.tile([128, 196], BF16)
        nc.sync.dma_start(wdecb[:], ins["wdecb"][:])
        wdecf = const.tile([DEC, 6], F32)
        nc.sync.dma_start(wdecf[:], ins["wdecf"][:])

        Gi = sum(g for (_, g) in meta["ioff"].values())
        Gf = sum(g for (_, g) in meta["foff"].values())
        idxp = ctx.enter_context(tc.tile_pool(name="idx", bufs=1))
        ipack = idxp.tile([128, Gi], I16)
        nc.sync.dma_start(ipack[:], ins["ipack"][:])
        fpack = idxp.tile([128, Gf], F32)
        nc.sync.dma_start(fpack[:], ins["fpack"][:])

        def islc(name):
            o, g = meta["ioff"][name]
            return ipack[:, o:o + g]

        def fslc(name):
            o, g = meta["foff"][name]
            return fpack[:, o:o + g]

        dram = ctx.enter_context(tc.tile_pool(name="dram", bufs=1, space="DRAM"))
        hbounce = [dram.tile([lv[l]["S"], H], BF16, tag=f"hb{l}", name=f"hb{l}")
                   for l in range(3)]
        htab = [dram.tile([C * lv[l]["S"], H], BF16, tag=f"ht{l}", name=f"ht{l}")
                for l in range(3)]
        ebounce = [dram.tile([lv[l]["S"], ELEM], BF16, tag=f"eb{l}", name=f"eb{l}")
                   for l in (1, 2)]
        etab = [dram.tile([C * lv[l]["S"], ELEM], BF16, tag=f"et{l}", name=f"et{l}")
                for l in (1, 2)]

        def allgather(bounce, tab):
            nc.gpsimd.collective_compute(
                "AllGather", ALU.bypass, replica_groups=rg,
                ins=[bounce.opt()], outs=[tab.opt()])

        def conv_passes(l, table):
            """passes for level-l conv gathers against `table` [N, ELEM]."""
            ps = []
            for pi, (blo, bhi) in enumerate(lv[l]["buckets"]):
                nm = f"e{l}p{pi}"
                ps.append({
                    "tab": table[blo:bhi, :],
                    "ei": islc(nm), "ed": fslc(nm), "av": None,
                    "ng": lv[l]["ngs"][pi], "accum": pi > 0,
                })
            return ps

        def wslc(l):
            Lf, Lb = l * WPF_L, l * WPB_L
            return dict(
                b1=wpf[:, Lf:Lf + 1], b2=wpf[:, Lf + 1:Lf + 2],
                linb=wpf[0:F3, Lf + 2:Lf + 3],
                wrel1=wpf[0:FIN, Lf + 3:Lf + 131],
                wroot1=wpf[0:FIN, Lf + 131:Lf + 259],
                wrel2=wpb[:, Lb:Lb + 128], wroot2=wpb[:, Lb + 128:Lb + 256],
                linw=wpb[:, Lb + 256:Lb + 352],
            )

        decp = ctx.enter_context(tc.tile_pool(name="dec", bufs=1))
        e0T = decp.tile([F3, S0], BF16, tag="e0T")

        with tc.tile_pool(name="acts", bufs=1) as actp:
            h1Ts = [actp.tile([H, lv[l]["S"]], BF16, tag=f"h1T{l}", name=f"h1T{l}")
                    for l in range(3)]

            # ---- phase A: conv1 on all levels; write + allgather h1 tables --
            with tc.tile_pool(name="pA", bufs=1) as pA:
                for l in range(3):
                    S = lv[l]["S"]
                    W = wslc(l)
                    aggF = pA.tile([FIN, S0], F32, tag="aggF", name="aggF")
                    xT = pA.tile([FIN, S0], F32, tag="xT", name="xT")
                    nc.sync.dma_start(xT[:, :S], ins[f"xT{l}"][:])
                    _seg_agg(tc, nc, f"c1l{l}", conv_passes(l, ins[f"xtab{l}"][:, :]),
                             S, FIN, aggF, iota[:], KCH)
                    _dense2(tc, nc, f"d1l{l}", aggF, xT, W["wrel1"], W["wroot1"],
                            W["b1"], FIN, H, S, AF.Relu, h1Ts[l])
                    _write_rows(tc, nc, f"h1l{l}", h1Ts[l], H, S, hbounce[l],
                                identb, BF16)
                    allgather(hbounce[l], htab[l])

            # ---- phase B: conv2 + lin on levels 1,2; allgather e tables ----
            for l in (1, 2):
                S = lv[l]["S"]
                W = wslc(l)
                aggH = actp.tile([H, S0], BF16, tag="aggH", name="aggH")
                h2T = actp.tile([H, S0], BF16, tag="h2T", name="h2T")
                eT = actp.tile([F3, lv[1]["S"]], BF16, tag="eT", name="eT")
                _seg_agg(tc, nc, f"c2l{l}", conv_passes(l, htab[l][:, :]),
                         S, H, aggH, iota[:], KCH)
                _dense2(tc, nc, f"d2l{l}", aggH, h1Ts[l], W["wrel2"], W["wroot2"],
                        W["b2"], H, H, S, AF.Relu, h2T)
                _dense2(tc, nc, f"linl{l}", h2T, None, W["linw"], None,
                        W["linb"], H, F3, S, AF.Identity, eT)
                _write_rows(tc, nc, f"el{l}", eT, F3, S, ebounce[l - 1],
                            identb, BF16, width=ELEM)
                allgather(ebounce[l - 1], etab[l - 1])

            # ---- phase C: conv2 + lin on level 0 ----
            W = wslc(0)
            aggH = actp.tile([H, S0], BF16, tag="aggH", name="aggH")
            h2T = actp.tile([H, S0], BF16, tag="h2T", name="h2T")
            _seg_agg(tc, nc, "c2l0", conv_passes(0, htab[0][:, :]),
                     S0, H, aggH, iota[:], KCH)
            _dense2(tc, nc, "d2l0", aggH, h1Ts[0], W["wrel2"], W["wroot2"],
                    W["b2"], H, H, S0, AF.Relu, h2T)
            _dense2(tc, nc, "linl0", h2T, None, W["linw"], None,
                    W["linb"], H, F3, S0, AF.Identity, e0T)

        # ---- phase D: interpolation onto fine level ----
        iTs = []
        for m in (1, 2):
            iT = decp.tile([F3, S0], BF16, tag=f"i{m}T", name=f"i{m}T")
            nm = f"a{m}p0"
            _seg_agg(tc, nc, f"a{m}", [{
                "tab": etab[m - 1][:, :], "ei": islc(nm), "ed": fslc(nm),
                "av": fslc(nm + "v"), "ng": meta["interp"][m - 1]["ngs"][0],
                "accum": False,
            }], S0, F3, iT, iota[:], KCH)
            iTs.append(iT)

        # ---- phase E: decoders ----
        srcs = [e0T, iTs[0], iTs[1]]
        with tc.tile_pool(name="dz", bufs=2) as zp, \
             tc.tile_pool(name="dps", bufs=2, space="PSUM") as pp, \
             tc.tile_pool(name="dos", bufs=2) as op_:
            for n0 in range(0, S0, NCHUNK):
                w = min(NCHUNK, S0 - n0)
                for b in range(3):
                    ps = pp.tile([DEC, NCHUNK], F32, tag="zps", name="zps")
                    for j in range(3):
                        nc.tensor.matmul(
                            ps[:, :w],
                            lhsT=wdecb[32 * b:32 * b + 32, 64 * j:64 * j + 64],
                            rhs=srcs[j][32 * b:32 * b + 32, n0:n0 + w],
                            start=(j == 0), stop=(j == 2))
                    b0c = wdecf[0:DEC, b:b + 1]
                    r = zp.tile([DEC, NCHUNK], BF16, tag="r", name="r")
                    mm = zp.tile([DEC, NCHUNK], BF16, tag="m", name="m")
                    nc.vector.tensor_scalar(out=r[:, :w], in0=ps[:, :w], scalar1=b0c,
                                            scalar2=0.0, op0=ALU.add, op1=ALU.max)
                    nc.vector.tensor_scalar(out=mm[:, :w], in0=ps[:, :w], scalar1=b0c,
                                            scalar2=0.0, op0=ALU.add, op1=ALU.min)
                    nc.scalar.activation(mm[:, :w], mm[:, :w], AF.Exp)
                    nc.vector.tensor_tensor(out=r[:, :w], in0=r[:, :w], in1=mm[:, :w],
                                            op=ALU.add)
                    pso = pp.tile([1, NCHUNK], F32, tag="ops", name="ops")
                    nc.tensor.matmul(pso[:, :w], lhsT=wdecb[0:DEC, 192 + b:193 + b],
                                     rhs=r[:, :w], start=True, stop=True)
                    orow = op_.tile([1, NCHUNK], F32, tag="orow", name="orow")
                    nc.scalar.activation(orow[:, :w], pso[:, :w], AF.Identity,
                                         bias=wdecf[0:1, 3 + b:4 + b])
                    nc.sync.dma_start(outs["out"][b:b + 1, n0:n0 + w], orow[:, :w])


# --------------------------------------------------------------------------
# driver
# --------------------------------------------------------------------------

_CACHE = {}


def _build(meta, in_map0):
    key = repr(meta)
    if key in _CACHE:
        return _CACHE[key]
    import time as _time
    _t0 = _time.time()
    nc = bacc.Bacc("TRN2", num_devices=C, debug=False)
    ins = {}
    for name, arr in in_map0.items():
        ins[name] = nc.dram_tensor(name, list(arr.shape),
                                   mybir.dt.from_np(arr.dtype),
                                   kind="ExternalInput").ap()
    out = nc.dram_tensor("out", [3, meta["levels"][0]["S"]], F32,
                         kind="ExternalOutput").ap()
    with tile.TileContext(nc) as tc:
        emit(tc, {"out": out}, ins, meta)
    print(f"[kernel] tile trace+schedule: {_time.time() - _t0:.1f}s", flush=True)
    nc.compile()
    print(f"[kernel] bass compile done: {_time.time() - _t0:.1f}s", flush=True)
    _CACHE[key] = nc
    return nc


def _ensure_axon_ntff_hook():
    """bass_utils wants antenv.axon_hooks for NTFF profiling under axon; some
    images lack it. Install a shim backed by trn_agent_boot's ctypes hook."""
    try:
        from antenv.axon_hooks import get_axon_ntff_profile_hook  # noqa: F401
        return True
    except Exception:
        pass
    try:
        import types
        if "/root/.axon_site" not in sys.path and os.path.isdir("/root/.axon_site"):
            sys.path.insert(0, "/root/.axon_site")
        from trn_agent_boot import trn_boot
        hook = trn_boot._ntff_profile_via_ctypes("/opt/axon/libaxon_pjrt.so")
        mod = types.ModuleType("antenv.axon_hooks")
        mod.get_axon_ntff_profile_hook = lambda: hook
        mod.set_axon_ntff_profile_hook = lambda h: None
        try:
            import antenv  # noqa: F401
        except Exception:
            pkg = types.ModuleType("antenv")
            pkg.__path__ = []
            sys.modules["antenv"] = pkg
        sys.modules["antenv.axon_hooks"] = mod
        return True
    except Exception as e:
        print(f"[kernel] ntff hook shim failed: {e}")
        return False


def kernel(**inputs):
    meta, in_maps = preprocess(inputs)
    nc = _build(meta, in_maps[0])
    trace = os.environ.get("KERNEL_TRACE", "0") == "1"
    if trace:
        trace = _ensure_axon_ntff_hook()
    try:
        res = bass_utils.run_bass_kernel_spmd(
            nc, in_maps, core_ids=list(range(C)), trace=trace)
    except Exception:
        if not trace:
            raise
        print("[kernel] traced run failed; retrying without trace")
        res = bass_utils.run_bass_kernel_spmd(
            nc, in_maps, core_ids=list(range(C)), trace=False)
    if trace and res.exec_time_ns is not None:
        print(f"HW exec time: {res.exec_time_ns} ns")
    S0 = meta["levels"][0]["S"]
    N0 = meta["levels"][0]["N"]
    out = np.empty((N0, 3, 1), np.float32)
    for c in range(C):
        lo = c * S0
        hi = min(N0, lo + S0)
        out[lo:hi, :, 0] = res.results[c]["out"][:, : hi - lo].T
    return out
